# revision 5
# baseline (speedup 1.0000x reference)
"""Trainium2 Bass kernel for nn_Loss_9749575762182.

Computes two scalar losses over (8192, 2048) fp32 tensors:
  wmse = mean((weight[:,None] * (target - input))**2)
  wcl  = mean(|(st*ln(tp+eps) + (1-st)*ln(1-tp+eps)) * obrT|)

Strategy: data-parallel over the row axis across 8 NeuronCores
(1024 rows each). Each core streams its 5 x 8MB tensor slices through
SBUF in eight [128, 2048] tiles, producing per-partition partial sums;
the tiny [128, 20] partials land back in DRAM and the host finishes
the reduction in float64.

The kernel is HBM-bound: 40MB/core over ~358GB/s peak = 112us floor;
~330GB/s (121us) is the observed practical stream rate. Evolution:
  v1 155.7us: 8 tiles, coarse tail (a full-2048 serial chain after the
      last byte) cost ~24us past the end of streaming.
  v2 143.1us: 18 column-split chunks (1024/512 wide). Short tail, but
      2-4KB DMA descriptors + 2x the dispatches/semaphores slowed the
      stream by ~4us and added ~1us stalls at chunk seams.
  v3: loads stay full [128, 2048] tiles (1MB contiguous, 8KB descriptor
      rows = best DMA efficiency, 40 dispatches total), while COMPUTE
      on the last tile is split into 1024+512+512 column passes so the
      post-stream serial chain is ~512-wide (~4us). Other wins kept:
      - First ACT instruction is an Ln touch: Bacc's
        insert_act_table_loads picks act-func-set 5 (ln+square+abs+copy)
        once, instead of set 0 + a 1283ns reload at the first real Ln.
      - mse/cl partials accumulate into ONE [128, 20] tile; the final
        store is issued from the ACT engine right after the last accum
        (ACT program order: no cross-engine semaphore, HWDGE latency).
      - Loads issue q (target_pre) first so the ACT ln chain starts
        as early as possible; per-tile compute runs l1/l2 first.

Per tile the math keeps the Vector and Scalar engines well under the
~14.8us/tile DMA budget:
  ACT: l1 = Ln(tp + eps)          (bias/scale fold the affine into the LUT)
  ACT: l2 = Ln(-tp + (1+eps))
  DVE: diff = target - input                     (tensor_tensor sub)
  ACT: Square(diff * w)  + accum -> mse partial  (scale = per-partition w)
  DVE: d = l1 - l2 ; m = st * d ; b = m + l2 ; po = b * obrT
  ACT: Abs(po) + accum -> cl partial

Hard-won environment notes (axon-tunneled trn2, this toolchain):
  - Build on bacc.Bacc() and call nc.finalize() before run_bass_via_pjrt;
    raw bass.Bass() BIR fails walrus ("Reg has not been allocated"), and
    without Bacc's generate_event_semaphores pass any instruction with
    >1 semaphore wait dies in codegen ("Too many sync wait commands").
  - tensor_tensor_reduce compiles + simulates fine but faults on real HW
    via the PJRT path; ACT Abs with accum_out replaces it.
  - Big loads go through nc.sync.dma_start (HW-DGE, fans out across HW
    queues): all-gpsimd SWDGE funnels through ONE dynamic queue
    (~216 GB/s ceiling observed -> 185us); HW-DGE gets 153us.
  - The CoreV3 ISA allows one sync-wait per instruction, and Tile
    doesn't split excess waits for free. Discipline: every instruction
    may depend on at most ONE foreign semaphore; tiny "touch" ops
    consume extra waits so the real consumers inherit them via engine
    program order / already-observed clocks.
"""

import os
import sys

if "/opt/trn_rl_repo" not in sys.path:
    sys.path.insert(0, "/opt/trn_rl_repo")

import numpy as np

N, D = 8192, 2048
NCORES = 8
ROWS = N // NCORES  # rows per core
P = 128             # SBUF partitions
NT = ROWS // P      # row-blocks per core (8)
EPS = 1e-10

# compute passes: one full-width pass for tiles 0..6, then a tapered
# tail (1024 + 512 + 512) on tile 7. 10 accum columns per loss.
NPASS = NT - 1 + 3

_CACHE = {}


def build(rows=ROWS, d=D, bufs=3):
    import concourse.bacc as bacc
    import concourse.tile as tile
    from concourse import mybir

    f32 = mybir.dt.float32
    ACTF = mybir.ActivationFunctionType

    nc = bacc.Bacc()
    inp = nc.dram_tensor("input", [rows, d], f32, kind="ExternalInput")
    tgt = nc.dram_tensor("target", [rows, d], f32, kind="ExternalInput")
    wgt = nc.dram_tensor("weight", [rows], f32, kind="ExternalInput")
    st = nc.dram_tensor("sub_target", [rows, d], f32, kind="ExternalInput")
    tp = nc.dram_tensor("target_pre", [rows, d], f32, kind="ExternalInput")
    ob = nc.dram_tensor("sub_obrT", [rows, d], f32, kind="ExternalInput")
    out = nc.dram_tensor("partials", [P, 2 * NPASS], f32, kind="ExternalOutput")

    inp_t = inp.rearrange("(t p) d -> t p d", p=P)
    tgt_t = tgt.rearrange("(t p) d -> t p d", p=P)
    st_t = st.rearrange("(t p) d -> t p d", p=P)
    tp_t = tp.rearrange("(t p) d -> t p d", p=P)
    ob_t = ob.rearrange("(t p) d -> t p d", p=P)
    wgt_t = wgt.rearrange("(t p) -> p t", p=P)

    with tile.TileContext(nc) as tc:
        with (
            tc.tile_pool(name="singles", bufs=1) as singles,
            tc.tile_pool(name="in_p", bufs=2) as in_p,
            tc.tile_pool(name="tgt_p", bufs=bufs) as tgt_p,
            tc.tile_pool(name="st_p", bufs=bufs) as st_p,
            tc.tile_pool(name="tp_p", bufs=2) as tp_p,
            tc.tile_pool(name="ob_p", bufs=bufs) as ob_p,
            tc.tile_pool(name="l1_p", bufs=2) as l1_p,
            tc.tile_pool(name="l2_p", bufs=2) as l2_p,
            tc.tile_pool(name="diff_p", bufs=2) as diff_p,
            tc.tile_pool(name="sq_p", bufs=1) as sq_p,
            tc.tile_pool(name="po_p", bufs=2) as po_p,
        ):
            w_cols = singles.tile([P, NT], f32)
            nc.gpsimd.dma_start(out=w_cols, in_=wgt_t)
            # mse partials in cols[:, :NPASS], cl partials in cols[:, NPASS:].
            # One tile, only ever touched by ACT -> single in-order stream.
            cols = singles.tile([P, 2 * NPASS], f32)
            eps_b = singles.tile([P, 1], f32)
            nc.vector.memset(eps_b, EPS)
            one_eps_b = singles.tile([P, 1], f32)
            nc.vector.memset(one_eps_b, 1.0 + EPS)
            zero_b = singles.tile([P, 1], f32)
            nc.vector.memset(zero_b, 0.0)

            touch_d = singles.tile([P, 1], f32)
            atouch_d = singles.tile([P, 1], f32)
            # First ACT instruction is an Ln: forces act-func-set 5
            # (ln+square+abs+copy) in one table load. zero_b is the last
            # DVE memset, so one wait covers all three memsets.
            nc.scalar.activation(
                out=atouch_d, in_=zero_b, func=ACTF.Ln, bias=zero_b, scale=1.0
            )
            nc.scalar.activation(
                out=atouch_d, in_=w_cols[:, 0:1], func=ACTF.Ln, bias=zero_b, scale=1.0
            )  # waits w_cols DMA on ACT

            def loads(t):
                q = tp_p.tile([P, d], f32, name="q")
                nc.sync.dma_start(out=q, in_=tp_t[t])
                x = in_p.tile([P, d], f32, name="x")
                nc.sync.dma_start(out=x, in_=inp_t[t])
                g = tgt_p.tile([P, d], f32, name="g")
                nc.sync.dma_start(out=g, in_=tgt_t[t])
                s = st_p.tile([P, d], f32, name="s")
                nc.sync.dma_start(out=s, in_=st_t[t])
                o = ob_p.tile([P, d], f32, name="o")
                nc.sync.dma_start(out=o, in_=ob_t[t])
                return q, x, g, s, o

            # ---- tiles 0..6: one full-width pass each
            for t in range(NT - 1):
                q, x, g, s, o = loads(t)
                wc = w_cols[:, t : t + 1]

                l1 = l1_p.tile([P, d], f32, name="l1")
                nc.scalar.activation(out=l1, in_=q, func=ACTF.Ln, bias=eps_b, scale=1.0)
                l2 = l2_p.tile([P, d], f32, name="l2")
                nc.scalar.activation(
                    out=l2, in_=q, func=ACTF.Ln, bias=one_eps_b, scale=-1.0
                )

                nc.vector.tensor_copy(touch_d, x[:, 0:1])  # consume x-DMA wait
                diff = diff_p.tile([P, d], f32, name="diff")
                nc.vector.tensor_sub(diff, g, x)  # waits only g-DMA
                sq = sq_p.tile([P, d], f32, name="sq")
                nc.scalar.activation(
                    out=sq,
                    in_=diff,
                    func=ACTF.Square,
                    bias=zero_b,
                    scale=wc,
                    accum_out=cols[:, t : t + 1],
                )

                nc.vector.tensor_sub(l1, l1, l2)  # waits ACT l2 (covers l1)
                nc.vector.tensor_mul(s, s, l1)    # waits s-DMA
                nc.vector.tensor_add(l2, s, l2)   # no new wait
                po = po_p.tile([P, d], f32, name="po")
                nc.vector.tensor_mul(po, l2, o)   # waits o-DMA
                ab = sq_p.tile([P, d], f32, name="ab")
                nc.scalar.activation(
                    out=ab,
                    in_=po,
                    func=ACTF.Abs,
                    bias=zero_b,
                    scale=1.0,
                    accum_out=cols[:, NPASS + t : NPASS + t + 1],
                )

            # ---- tile 7: tapered compute (1024 + 512 + 512 column passes)
            # over full-width loads, engine-ordered so the post-stream
            # chain is as short as possible:
            #   ACT: (l1,l2) x3, sq x3, ab x3, store
            #   DVE: touch, diff x3, (sub,mul,add) x3, po-mul x3
            t = NT - 1
            q, x, g, s, o = loads(t)
            wc = w_cols[:, t : t + 1]
            spans = [(0, 1024), (1024, 1536), (1536, 2048)]
            tags = ["", "a", "b"]

            l1s, l2s, diffs, pos = [], [], [], []
            for k, (c0, c1) in enumerate(spans):
                cw = c1 - c0
                l1k = l1_p.tile([P, cw], f32, name="l1")
                nc.scalar.activation(
                    out=l1k, in_=q[:, c0:c1], func=ACTF.Ln, bias=eps_b, scale=1.0
                )
                l2k = l2_p.tile([P, cw], f32, name="l2")
                nc.scalar.activation(
                    out=l2k, in_=q[:, c0:c1], func=ACTF.Ln, bias=one_eps_b, scale=-1.0
                )
                l1s.append(l1k)
                l2s.append(l2k)

            nc.vector.tensor_copy(touch_d, x[:, 0:1])  # consume x-DMA wait
            for k, (c0, c1) in enumerate(spans):
                cw = c1 - c0
                dk = diff_p.tile([P, cw], f32, name="diff")
                nc.vector.tensor_sub(dk, g[:, c0:c1], x[:, c0:c1])
                diffs.append(dk)
            for k in range(3):
                sqk = sq_p.tile([P, spans[k][1] - spans[k][0]], f32, name="sq")
                nc.scalar.activation(
                    out=sqk,
                    in_=diffs[k],
                    func=ACTF.Square,
                    bias=zero_b,
                    scale=wc,
                    accum_out=cols[:, t + k : t + k + 1],
                )
            for k, (c0, c1) in enumerate(spans):
                nc.vector.tensor_sub(l1s[k], l1s[k], l2s[k])
                nc.vector.tensor_mul(s[:, c0:c1], s[:, c0:c1], l1s[k])
                nc.vector.tensor_add(l2s[k], s[:, c0:c1], l2s[k])
            for k, (c0, c1) in enumerate(spans):
                cw = c1 - c0
                pok = po_p.tile([P, cw], f32, name="po", tag=tags[k],
                                bufs=None if k == 0 else 1)
                nc.vector.tensor_mul(pok, l2s[k], o[:, c0:c1])  # k=0 waits o-DMA
                pos.append(pok)
            for k in range(3):
                abk = sq_p.tile([P, spans[k][1] - spans[k][0]], f32, name="ab")
                nc.scalar.activation(
                    out=abk,
                    in_=pos[k],
                    func=ACTF.Abs,
                    bias=zero_b,
                    scale=1.0,
                    accum_out=cols[:, NPASS + t + k : NPASS + t + k + 1],
                )

            # ACT-issued HWDGE store: in ACT program order after the last
            # Abs, so it needs no cross-engine semaphore.
            nc.scalar.dma_start(out=out[:, 0 : 2 * NPASS], in_=cols)
    return nc


def _get_nc():
    if "nc" not in _CACHE:
        nc = build()
        nc.finalize()  # runs Bacc's passes (event-sem wait splitting, regalloc)
        _CACHE["nc"] = nc
    return _CACHE["nc"]


def _install_profile_hook():
    """Register the NTFF profile hook that this container's stripped antenv
    lacks: a ctypes bridge into libaxon_pjrt.so (same ABI trn_boot.py uses).
    Only needed for trace=True runs."""
    if "antenv.axon_hooks" in sys.modules:
        return
    import contextlib
    import ctypes
    import types

    so_path = "/opt/axon/libaxon_pjrt.so"
    lib = ctypes.CDLL(so_path)
    if not hasattr(lib, "axon_start_nrt_profile"):
        return
    lib.axon_start_nrt_profile.argtypes = [
        ctypes.POINTER(ctypes.c_int64),
        ctypes.c_size_t,
    ]
    lib.axon_start_nrt_profile.restype = ctypes.c_int64
    lib.axon_stop_nrt_profile.argtypes = [ctypes.c_char_p]
    lib.axon_stop_nrt_profile.restype = ctypes.c_int64

    @contextlib.contextmanager
    def _hook(output_dir, device_ids):
        import jax

        jax.devices()
        if device_ids:
            ids = (ctypes.c_int64 * len(device_ids))(*device_ids)
            rc = lib.axon_start_nrt_profile(ids, len(device_ids))
        else:
            rc = lib.axon_start_nrt_profile(None, 0)
        if rc != 0:
            raise RuntimeError(f"axon_start_nrt_profile rc={rc}")
        try:
            yield
        finally:
            n = lib.axon_stop_nrt_profile(str(output_dir).encode())
            print(f"profile: {n} file(s) written to {output_dir}")

    mod = types.ModuleType("antenv.axon_hooks")
    mod.get_axon_ntff_profile_hook = lambda: _hook
    sys.modules["antenv.axon_hooks"] = mod


def kernel(**inputs):
    from concourse.bass_utils import run_bass_kernel_spmd

    nc = _get_nc()
    names = ["input", "target", "weight", "sub_target", "target_pre", "sub_obrT"]
    arrs = {k: np.ascontiguousarray(np.asarray(inputs[k], dtype=np.float32)) for k in names}
    in_maps = []
    for c in range(NCORES):
        sl = slice(c * ROWS, (c + 1) * ROWS)
        in_maps.append({k: np.ascontiguousarray(v[sl]) for k, v in arrs.items()})

    trace = os.environ.get("BASS_KERNEL_PROFILE", "0") == "1"
    if trace:
        _install_profile_hook()
    res = run_bass_kernel_spmd(nc, in_maps, list(range(NCORES)), trace=trace)

    mse_sum = 0.0
    cl_sum = 0.0
    for r in res.results:
        part = np.asarray(r["partials"], dtype=np.float64)
        mse_sum += part[:, :NPASS].sum()
        cl_sum += part[:, NPASS:].sum()
    tot = float(N) * float(D)
    if trace and res.exec_time_ns is not None:
        print(f"HW exec time: {res.exec_time_ns} ns")
    return (
        np.asarray(np.float32(mse_sum / tot)),
        np.asarray(np.float32(cl_sum / tot)),
    )


# revision 7
# speedup vs baseline: 1.0841x; 1.0841x over previous
"""Trainium2 Bass kernel for nn_Loss_9749575762182.

Computes two scalar losses over (8192, 2048) fp32 tensors:
  wmse = mean((weight[:,None] * (target - input))**2)
  wcl  = mean(|(st*ln(tp+eps) + (1-st)*ln(1-tp+eps)) * obrT|)

Strategy: data-parallel over the row axis across 8 NeuronCores
(1024 rows each). Each core streams its 5 x 8MB tensor slices through
SBUF in eight [128, 2048] tiles, producing per-partition partial sums;
the tiny [128, 20] partials land back in DRAM and the host finishes
the reduction in float64.

The kernel is HBM-bound: 40MB/core over ~358GB/s peak = 112us floor;
~330GB/s (121us) is the observed practical stream rate. Evolution:
  v1 155.7us: 8 tiles, coarse tail (a full-2048 serial chain after the
      last byte) cost ~24us past the end of streaming.
  v2 143.1us: 18 column-split chunks (1024/512 wide). Short tail, but
      2-4KB DMA descriptors + 2x the dispatches/semaphores slowed the
      stream by ~4us and added ~1us stalls at chunk seams.
  v3: loads stay full [128, 2048] tiles (1MB contiguous, 8KB descriptor
      rows = best DMA efficiency, 40 dispatches total), while COMPUTE
      on the last tile is split into 1024+512+512 column passes so the
      post-stream serial chain is ~512-wide (~4us). Other wins kept:
      - First ACT instruction is an Ln touch: Bacc's
        insert_act_table_loads picks act-func-set 5 (ln+square+abs+copy)
        once, instead of set 0 + a 1283ns reload at the first real Ln.
      - mse/cl partials accumulate into ONE [128, 20] tile; the final
        store is issued from the ACT engine right after the last accum
        (ACT program order: no cross-engine semaphore, HWDGE latency).
      - Loads issue q (target_pre) first so the ACT ln chain starts
        as early as possible; per-tile compute runs l1/l2 first.

Per tile the math keeps the Vector and Scalar engines well under the
~14.8us/tile DMA budget:
  ACT: l1 = Ln(tp + eps)          (bias/scale fold the affine into the LUT)
  ACT: l2 = Ln(-tp + (1+eps))
  DVE: diff = target - input                     (tensor_tensor sub)
  ACT: Square(diff * w)  + accum -> mse partial  (scale = per-partition w)
  DVE: d = l1 - l2 ; m = st * d ; b = m + l2 ; po = b * obrT
  ACT: Abs(po) + accum -> cl partial

Hard-won environment notes (axon-tunneled trn2, this toolchain):
  - Build on bacc.Bacc() and call nc.finalize() before run_bass_via_pjrt;
    raw bass.Bass() BIR fails walrus ("Reg has not been allocated"), and
    without Bacc's generate_event_semaphores pass any instruction with
    >1 semaphore wait dies in codegen ("Too many sync wait commands").
  - tensor_tensor_reduce compiles + simulates fine but faults on real HW
    via the PJRT path; ACT Abs with accum_out replaces it.
  - Big loads go through nc.sync.dma_start (HW-DGE, fans out across HW
    queues): all-gpsimd SWDGE funnels through ONE dynamic queue
    (~216 GB/s ceiling observed -> 185us); HW-DGE gets 153us.
  - The CoreV3 ISA allows one sync-wait per instruction, and Tile
    doesn't split excess waits for free. Discipline: every instruction
    may depend on at most ONE foreign semaphore; tiny "touch" ops
    consume extra waits so the real consumers inherit them via engine
    program order / already-observed clocks.
"""

import os
import sys

if "/opt/trn_rl_repo" not in sys.path:
    sys.path.insert(0, "/opt/trn_rl_repo")

import numpy as np

N, D = 8192, 2048
NCORES = 8
ROWS = N // NCORES  # rows per core
P = 128             # SBUF partitions
NT = ROWS // P      # row-blocks per core (8)
EPS = 1e-10

# accumulator columns: 9 mse (7 full tiles + 2 half passes on tile 7),
# 16 cl (two per tile/pass: sum(c1*l1), sum(c2*l2)).
MSE_COLS = NT - 1 + 2
CL_COLS = 2 * (NT - 1) + 2
NCOLS = MSE_COLS + CL_COLS

_CACHE = {}


def build(rows=ROWS, d=D):
    import concourse.bacc as bacc
    import concourse.tile as tile
    from concourse import mybir

    f32 = mybir.dt.float32
    ACTF = mybir.ActivationFunctionType
    ALU = mybir.AluOpType

    nc = bacc.Bacc()
    inp = nc.dram_tensor("input", [rows, d], f32, kind="ExternalInput")
    tgt = nc.dram_tensor("target", [rows, d], f32, kind="ExternalInput")
    wgt = nc.dram_tensor("weight", [rows], f32, kind="ExternalInput")
    st = nc.dram_tensor("sub_target", [rows, d], f32, kind="ExternalInput")
    tp = nc.dram_tensor("target_pre", [rows, d], f32, kind="ExternalInput")
    ob = nc.dram_tensor("sub_obrT", [rows, d], f32, kind="ExternalInput")
    out = nc.dram_tensor("partials", [P, NCOLS], f32, kind="ExternalOutput")

    inp_t = inp.rearrange("(t p) d -> t p d", p=P)
    tgt_t = tgt.rearrange("(t p) d -> t p d", p=P)
    st_t = st.rearrange("(t p) d -> t p d", p=P)
    tp_t = tp.rearrange("(t p) d -> t p d", p=P)
    ob_t = ob.rearrange("(t p) d -> t p d", p=P)
    wgt_t = wgt.rearrange("(t p) -> p t", p=P)

    with tile.TileContext(nc) as tc:
        with (
            tc.tile_pool(name="singles", bufs=1) as singles,
            tc.tile_pool(name="in_p", bufs=2) as in_p,
            tc.tile_pool(name="tgt_p", bufs=2) as tgt_p,
            tc.tile_pool(name="st_p", bufs=3) as st_p,
            tc.tile_pool(name="tp_p", bufs=2) as tp_p,
            tc.tile_pool(name="ob_p", bufs=3) as ob_p,
            tc.tile_pool(name="l1_p", bufs=2) as l1_p,
            tc.tile_pool(name="l2_p", bufs=2) as l2_p,
            tc.tile_pool(name="d_p", bufs=2) as d_p,
            tc.tile_pool(name="c1_p", bufs=2) as c1_p,
            tc.tile_pool(name="c2_p", bufs=2) as c2_p,
            tc.tile_pool(name="trash_p", bufs=1) as trash_p,
        ):
            w_cols = singles.tile([P, NT], f32)
            nc.gpsimd.dma_start(out=w_cols, in_=wgt_t)
            # per-partition accumulator columns: mse in [:MSE_COLS],
            # bce*ob sums (negated on host) in [MSE_COLS:]. Written only
            # by DVE accum_outs -> single in-order stream; SP stores it.
            cols = singles.tile([P, NCOLS], f32)
            eps_b = singles.tile([P, 1], f32)
            nc.vector.memset(eps_b, EPS)
            one_eps_b = singles.tile([P, 1], f32)
            nc.vector.memset(one_eps_b, 1.0 + EPS)
            zero_b = singles.tile([P, 1], f32)
            nc.vector.memset(zero_b, 0.0)
            w2 = singles.tile([P, NT], f32)
            nc.vector.tensor_mul(w2, w_cols, w_cols)  # waits w_cols DMA

            touch_d = singles.tile([P, 1], f32)
            atouch_d = singles.tile([P, 1], f32)
            # First ACT instruction is an Ln: loads act-func-set 5 once and
            # consumes the DVE-memset wait (zero_b is the last memset) so
            # later Lns only wait their q-DMA.
            nc.scalar.activation(
                out=atouch_d, in_=zero_b, func=ACTF.Ln, bias=zero_b, scale=1.0
            )

            mse_c = 0      # next mse accum column
            cl_c = MSE_COLS  # next cl accum column

            def lns(q, c0, c1):
                """l1 = ln(q+eps), l2 = ln(1+eps-q) on ACT."""
                cw = c1 - c0
                l1 = l1_p.tile([P, cw], f32, name="l1")
                nc.scalar.activation(
                    out=l1, in_=q[:, c0:c1], func=ACTF.Ln, bias=eps_b, scale=1.0
                )
                l2 = l2_p.tile([P, cw], f32, name="l2")
                nc.scalar.activation(
                    out=l2, in_=q[:, c0:c1], func=ACTF.Ln, bias=one_eps_b, scale=-1.0
                )
                return l1, l2

            def mse_pass(x, g, wc, c0, c1):
                """accum sum((w*(g-x))^2) via d then (d*w^2)*d."""
                nonlocal mse_c
                cw = c1 - c0
                dd = d_p.tile([P, cw], f32, name="dd")
                nc.vector.scalar_tensor_tensor(
                    dd, g[:, c0:c1], 0.0, x[:, c0:c1], ALU.bypass, ALU.subtract
                )
                tr = trash_p.tile([P, cw], f32, name="tr")
                nc.vector.scalar_tensor_tensor(
                    tr, dd, wc, dd, ALU.mult, ALU.mult,
                    accum_out=cols[:, mse_c : mse_c + 1],
                )
                mse_c += 1

            def cl_pass(s, o, l1, l2, c0, c1):
                """accum sum(bce*ob) = sum(c1*l1) + sum(c2*l2),
                c1 = st*ob, c2 = ob - c1. l1/l2 already column-sliced."""
                nonlocal cl_c
                cw = c1 - c0
                cc1 = c1_p.tile([P, cw], f32, name="cc1")
                nc.vector.scalar_tensor_tensor(
                    cc1, s[:, c0:c1], 0.0, o[:, c0:c1], ALU.bypass, ALU.mult
                )
                cc2 = c2_p.tile([P, cw], f32, name="cc2")
                nc.vector.scalar_tensor_tensor(
                    cc2, o[:, c0:c1], 0.0, cc1, ALU.bypass, ALU.subtract
                )
                tr = trash_p.tile([P, cw], f32, name="tr")
                nc.vector.scalar_tensor_tensor(
                    tr, cc1, 0.0, l1, ALU.bypass, ALU.mult,
                    accum_out=cols[:, cl_c : cl_c + 1],
                )
                tr2 = trash_p.tile([P, cw], f32, name="tr")
                nc.vector.scalar_tensor_tensor(
                    tr2, cc2, 0.0, l2, ALU.bypass, ALU.mult,
                    accum_out=cols[:, cl_c + 1 : cl_c + 2],
                )
                cl_c += 2

            # ---- tiles 0..6: full-width single pass
            for t in range(NT - 1):
                q = tp_p.tile([P, d], f32, name="q")
                nc.sync.dma_start(out=q, in_=tp_t[t])
                x = in_p.tile([P, d], f32, name="x")
                nc.sync.dma_start(out=x, in_=inp_t[t])
                g = tgt_p.tile([P, d], f32, name="g")
                nc.sync.dma_start(out=g, in_=tgt_t[t])
                s = st_p.tile([P, d], f32, name="s")
                nc.sync.dma_start(out=s, in_=st_t[t])
                o = ob_p.tile([P, d], f32, name="o")
                nc.sync.dma_start(out=o, in_=ob_t[t])

                l1, l2 = lns(q, 0, d)
                nc.vector.tensor_copy(touch_d, x[:, 0:1])  # consume x-DMA wait
                mse_pass(x, g, w2[:, t : t + 1], 0, d)     # dd waits g-DMA
                nc.vector.tensor_copy(touch_d, s[:, 0:1])  # consume s-DMA wait
                cl_pass(s, o, l1, l2, 0, d)  # cc1 waits o; STT1/2 wait ACT

            # ---- tile 7: loads reordered (s,o,q first; x,g split in half)
            # so the post-stream chain is two 1024-wide STTs + the store.
            t = NT - 1
            s = st_p.tile([P, d], f32, name="s")
            nc.sync.dma_start(out=s, in_=st_t[t])
            o = ob_p.tile([P, d], f32, name="o")
            nc.sync.dma_start(out=o, in_=ob_t[t])
            q = tp_p.tile([P, d], f32, name="q")
            nc.sync.dma_start(out=q, in_=tp_t[t])
            xh, gh = [], []
            for h in range(2):
                c0, c1 = h * (d // 2), (h + 1) * (d // 2)
                xk = in_p.tile([P, d // 2], f32, name="x", tag="xh", bufs=2)
                nc.sync.dma_start(out=xk, in_=inp_t[t][:, c0:c1])
                gk = tgt_p.tile([P, d // 2], f32, name="g", tag="gh", bufs=2)
                nc.sync.dma_start(out=gk, in_=tgt_t[t][:, c0:c1])
                xh.append(xk)
                gh.append(gk)

            l1, l2 = lns(q, 0, d)
            nc.vector.tensor_copy(touch_d, s[:, 0:1])  # consume s-DMA wait
            cl_pass(s, o, l1, l2, 0, d)
            for h in range(2):
                c0 = h * (d // 2)
                nc.vector.tensor_copy(touch_d, xh[h][:, 0:1])
                mse_pass(xh[h], gh[h], w2[:, t : t + 1], 0, d // 2)

            # SP-issued store: last in SP program order, waits only the
            # final DVE accum (cols has a single writer engine).
            nc.sync.dma_start(out=out[:, 0:NCOLS], in_=cols)
    return nc


def _get_nc():
    if "nc" not in _CACHE:
        nc = build()
        nc.finalize()  # runs Bacc's passes (event-sem wait splitting, regalloc)
        _CACHE["nc"] = nc
    return _CACHE["nc"]


def _install_profile_hook():
    """Register the NTFF profile hook that this container's stripped antenv
    lacks: a ctypes bridge into libaxon_pjrt.so (same ABI trn_boot.py uses).
    Only needed for trace=True runs."""
    if "antenv.axon_hooks" in sys.modules:
        return
    import contextlib
    import ctypes
    import types

    so_path = "/opt/axon/libaxon_pjrt.so"
    lib = ctypes.CDLL(so_path)
    if not hasattr(lib, "axon_start_nrt_profile"):
        return
    lib.axon_start_nrt_profile.argtypes = [
        ctypes.POINTER(ctypes.c_int64),
        ctypes.c_size_t,
    ]
    lib.axon_start_nrt_profile.restype = ctypes.c_int64
    lib.axon_stop_nrt_profile.argtypes = [ctypes.c_char_p]
    lib.axon_stop_nrt_profile.restype = ctypes.c_int64

    @contextlib.contextmanager
    def _hook(output_dir, device_ids):
        import jax

        jax.devices()
        if device_ids:
            ids = (ctypes.c_int64 * len(device_ids))(*device_ids)
            rc = lib.axon_start_nrt_profile(ids, len(device_ids))
        else:
            rc = lib.axon_start_nrt_profile(None, 0)
        if rc != 0:
            raise RuntimeError(f"axon_start_nrt_profile rc={rc}")
        try:
            yield
        finally:
            n = lib.axon_stop_nrt_profile(str(output_dir).encode())
            print(f"profile: {n} file(s) written to {output_dir}")

    mod = types.ModuleType("antenv.axon_hooks")
    mod.get_axon_ntff_profile_hook = lambda: _hook
    sys.modules["antenv.axon_hooks"] = mod


def kernel(**inputs):
    from concourse.bass_utils import run_bass_kernel_spmd

    nc = _get_nc()
    names = ["input", "target", "weight", "sub_target", "target_pre", "sub_obrT"]
    arrs = {k: np.ascontiguousarray(np.asarray(inputs[k], dtype=np.float32)) for k in names}
    in_maps = []
    for c in range(NCORES):
        sl = slice(c * ROWS, (c + 1) * ROWS)
        in_maps.append({k: np.ascontiguousarray(v[sl]) for k, v in arrs.items()})

    trace = os.environ.get("BASS_KERNEL_PROFILE", "0") == "1"
    if trace:
        _install_profile_hook()
    res = run_bass_kernel_spmd(nc, in_maps, list(range(NCORES)), trace=trace)

    mse_sum = 0.0
    cl_sum = 0.0
    for r in res.results:
        part = np.asarray(r["partials"], dtype=np.float64)
        mse_sum += part[:, :MSE_COLS].sum()
        cl_sum -= part[:, MSE_COLS:].sum()  # bce*ob <= 0: |.| = -(.)
    tot = float(N) * float(D)
    if trace and res.exec_time_ns is not None:
        print(f"HW exec time: {res.exec_time_ns} ns")
    return (
        np.asarray(np.float32(mse_sum / tot)),
        np.asarray(np.float32(cl_sum / tot)),
    )
